# revision 2
# baseline (speedup 1.0000x reference)
"""CTC loss kernel for Trainium2 (Bass/Tile), 8-core data-parallel.

Two-phase design:
  Phase 1 (gather): per batch row b, one-hot matmul E_b^T @ yT_b on the PE
    gathers extended-label channel probabilities into [state, time] tiles,
    stored to a DRAM scratch in a (group, b, slot, t)-contiguous layout so
    both the store and the reload use large DMA packets. y is pre-scaled on
    host: p'[t] = (y + eps) * 2^kt[t] (global integer drift schedule), bf16.
  Phase 2 (wavefront scan): partitions = 32 b x 4 time-chunks; wave w
    computes row s = w - 8g for chunk g as a hardware scan along time:
    state = (h + state) * p', h = c1*ring[w-1] + c2*ring[w-2].
    Numerical robustness: static per-(chunk,row) scales (VG) folded into
    the mixing tables, plus a per-row dynamic power-of-2 rescale at every
    chunk crossing (integer exponent kappa tracked per partition; extracted
    with f32 bit tricks - no scalar-engine tables). Chunk edges and kappa
    pass between groups via PE shift matmuls, batched 4 waves at a time.
Final: loss = -(ln(e127 + e128*2^d) + kappa127*ln2 - (V127+Ktot)*ln2).
"""
import numpy as np
import ml_dtypes
from contextlib import ExitStack

B_ALL, T, C, L = 256, 1024, 128, 64
S = 2 * L + 1            # 129
BL = B_ALL // 8          # 32 rows per core
G, CH = 4, 256
SKEW = 8
W = S + SKEW * (G - 1)   # 153 waves
NSLOT = W                # lp slots (slot = s + 8g)
KB = 4                   # crossing batch (waves per batch)
WP = ((W + KB - 1) // KB) * KB + KB   # padded table width (160)
EPS = 1e-7
LN2 = float(np.log(2.0))
SLOPE = 1.1
FBIAS = 12582912.0 + 127.0   # 0x4B400000 as f32 is 12582912; +127 exp bias

VG = [[74, 48, 33, 19, 10, 0, -9, -17, -25, -33, -40, -46, -51, -60, -64, -71, -75, -81, -84, -89, -94, -99, -103, -108, -112, -116, -119, -122, -126, -131, -134, -138, -141, -146, -149, -152, -154, -158, -161, -165, -167, -171, -173, -176, -179, -182, -184, -188, -191, -195, -197, -200, -202, -204, -206, -209, -212, -215, -216, -218, -220, -223, -225, -228, -229, -232, -233, -236, -238, -241, -242, -245, -245, -248, -249, -252, -253, -255, -256, -259, -260, -262, -263, -265, -267, -270, -270, -272, -273, -275, -276, -278, -279, -281, -282, -284, -285, -287, -288, -291, -292, -294, -295, -297, -297, -300, -300, -302, -303, -304, -305, -307, -308, -310, -310, -312, -312, -314, -314, -316, -317, -318, -319, -320, -321, -323, -322, -324, -324],
      [160, 117, 96, 81, 68, 52, 41, 27, 18, 6, -3, -12, -19, -28, -34, -43, -49, -58, -64, -71, -78, -86, -91, -97, -103, -110, -116, -122, -127, -135, -139, -145, -149, -156, -160, -165, -170, -175, -179, -184, -188, -193, -197, -201, -205, -210, -214, -219, -223, -227, -230, -234, -237, -240, -245, -250, -253, -257, -260, -265, -268, -271, -275, -278, -282, -285, -288, -293, -296, -299, -302, -305, -308, -312, -314, -318, -321, -324, -326, -330, -332, -336, -339, -341, -345, -347, -350, -354, -357, -360, -362, -366, -368, -371, -373, -377, -379, -382, -385, -388, -390, -393, -395, -398, -400, -402, -405, -408, -410, -412, -415, -418, -419, -421, -424, -427, -429, -431, -433, -437, -438, -440, -442, -444, -446, -448, -450, -452, -454],
      [249, 194, 169, 149, 133, 114, 98, 83, 71, 58, 48, 33, 25, 16, 6, -1, -11, -18, -28, -37, -45, -53, -60, -67, -76, -83, -89, -96, -103, -112, -118, -125, -131, -138, -145, -152, -156, -163, -169, -174, -180, -187, -192, -197, -202, -208, -213, -217, -223, -227, -233, -238, -243, -248, -253, -257, -263, -266, -271, -277, -281, -286, -290, -294, -298, -303, -306, -311, -315, -319, -324, -327, -332, -337, -341, -345, -349, -353, -357, -361, -364, -368, -372, -376, -379, -383, -387, -391, -394, -399, -402, -406, -409, -412, -415, -419, -422, -425, -428, -431, -435, -439, -442, -447, -450, -453, -456, -459, -462, -467, -469, -474, -476, -479, -482, -484, -488, -490, -494, -496, -499, -502, -506, -509, -512, -514, -517, -520, -523],
      [342, 279, 249, 216, 199, 180, 164, 145, 134, 121, 108, 92, 81, 68, 59, 43, 35, 22, 15, 4, -3, -13, -18, -28, -36, -45, -52, -63, -70, -77, -83, -94, -100, -108, -115, -124, -130, -140, -146, -152, -157, -166, -171, -179, -184, -193, -197, -205, -209, -217, -222, -229, -234, -240, -244, -250, -254, -262, -266, -272, -276, -283, -287, -294, -297, -303, -307, -313, -316, -322, -325, -332, -335, -340, -344, -351, -354, -360, -363, -370, -373, -378, -381, -387, -391, -397, -401, -406, -409, -414, -416, -423, -426, -431, -433, -439, -442, -448, -452, -456, -460, -464, -468, -472, -475, -481, -483, -487, -490, -494, -497, -502, -505, -510, -513, -517, -519, -524, -527, -531, -534, -538, -541, -546, -549, -553, -556, -560, -563]]
DG = [[0] * S,
      [-86, -69, -64, -61, -58, -52, -50, -45, -43, -39, -36, -35, -32, -32, -30, -28, -26, -23, -21, -19, -15, -13, -12, -12, -9, -6, -3, 0, 2, 4, 6, 7, 8, 10, 11, 13, 16, 17, 18, 20, 21, 22, 24, 24, 27, 28, 30, 31, 32, 32, 33, 33, 35, 36, 38, 40, 41, 42, 44, 47, 48, 48, 50, 51, 52, 53, 55, 57, 58, 58, 60, 61, 62, 63, 65, 66, 68, 69, 70, 71, 72, 73, 75, 76, 78, 78, 80, 82, 83, 84, 86, 87, 89, 89, 91, 93, 94, 96, 97, 97, 98, 99, 100, 101, 103, 102, 104, 106, 107, 108, 110, 111, 112, 112, 114, 115, 115, 115, 115, 115, 115, 115, 115, 115, 115, 115, 115, 115, 115],
      [-89, -76, -73, -69, -65, -62, -57, -56, -53, -52, -51, -45, -44, -44, -41, -43, -39, -40, -36, -34, -33, -32, -31, -29, -27, -27, -27, -27, -24, -22, -21, -20, -18, -18, -15, -13, -13, -12, -10, -11, -9, -6, -5, -3, -3, -2, -1, -2, 0, 1, 3, 5, 6, 7, 9, 8, 9, 9, 11, 12, 14, 15, 15, 16, 16, 18, 18, 18, 19, 20, 22, 22, 24, 26, 27, 27, 29, 29, 31, 31, 32, 33, 33, 34, 35, 36, 36, 36, 37, 39, 40, 40, 41, 41, 42, 42, 43, 43, 44, 43, 44, 45, 47, 48, 50, 51, 51, 52, 53, 55, 54, 56, 56, 58, 58, 58, 59, 59, 61, 59, 62, 62, 63, 64, 66, 66, 68, 68, 69],
      [-92, -86, -80, -67, -66, -66, -66, -62, -63, -63, -60, -58, -56, -52, -52, -44, -46, -40, -42, -41, -43, -41, -42, -39, -40, -39, -37, -33, -33, -35, -35, -31, -32, -29, -30, -27, -26, -24, -24, -21, -23, -21, -21, -18, -18, -15, -16, -13, -14, -10, -11, -9, -9, -7, -9, -8, -9, -4, -5, -5, -6, -3, -3, 0, -1, 0, 1, 2, 2, 3, 2, 5, 3, 3, 2, 6, 5, 7, 6, 9, 8, 10, 9, 12, 11, 14, 14, 16, 15, 15, 14, 17, 17, 19, 18, 20, 21, 23, 23, 25, 25, 25, 25, 26, 25, 28, 27, 27, 27, 27, 28, 29, 29, 31, 31, 32, 32, 34, 34, 36, 35, 36, 36, 37, 38, 39, 39, 40, 39]]

_KT_CUM = np.round(SLOPE * np.arange(T + 1)).astype(np.int64)
KT = np.diff(_KT_CUM)
KTOT = int(KT.sum())

_cache = {}


def _wave_gs(p, w):
    """(g, s) for partition p at wave w; s may be out of range (garbage)."""
    g = p // BL
    return g, w - SKEW * g


def _build_tables():
    """Static [128, WP] f32 tables for phase 2."""
    lam1 = np.zeros((128, WP), np.float32)
    lam2base = np.zeros((128, WP), np.float64)  # without allow (per-b applied later)
    scx = np.zeros((128, WP), np.float32)       # 2^(dV + D); 0 for g0/garbage
    adj = np.zeros((128, WP), np.float32)
    dini = np.zeros((128, WP), np.float32)      # 2^-D for real g>=1 crossings
    for p in range(128):
        g = p // BL
        for w in range(WP):
            s = w - SKEW * g
            real = 0 <= s <= S - 1
            if real and s >= 1:
                lam1[p, w] = 2.0 ** float(VG[g][s] - VG[g][s - 1])
            if real and s >= 2:
                lam2base[p, w] = 2.0 ** float(VG[g][s] - VG[g][s - 2])
            if real and g >= 1:
                ev = VG[g][s] - VG[g - 1][s] + DG[g][s]
                ev = max(-120, min(120, ev))
                scx[p, w] = 2.0 ** float(ev)
                adj[p, w] = -127.0
                dini[p, w] = 2.0 ** float(-DG[g][s] - 1)
            else:
                # g0 or garbage: v = max(0, 2^-40) -> fb encodes k=-40;
                # adj makes kappa_new = kappa_src (chains through).
                scx[p, w] = 0.0
                adj[p, w] = -87.0
                dini[p, w] = 0.0
    return lam1, lam2base, scx, adj, dini


def _build_nc():
    import concourse.bass as bass
    import concourse.tile as tile
    from concourse import bacc, mybir
    f32 = mybir.dt.float32
    bf16 = mybir.dt.bfloat16
    i32 = mybir.dt.int32
    Alu = mybir.AluOpType

    nc = bacc.Bacc("TRN2", target_bir_lowering=False, debug=False)
    ytd = nc.dram_tensor("yt", [BL, 128, T], bf16, kind="ExternalInput").ap()
    earr = nc.dram_tensor("earr", [128, BL * 128], bf16, kind="ExternalInput").ap()
    shEd = nc.dram_tensor("shE", [128, 128], bf16, kind="ExternalInput").ap()
    shKd = nc.dram_tensor("shK", [128, 128], f32, kind="ExternalInput").ap()
    lam1d = nc.dram_tensor("lam1", [128, WP], f32, kind="ExternalInput").ap()
    lam2d = nc.dram_tensor("lam2", [128, WP], f32, kind="ExternalInput").ap()
    scxd = nc.dram_tensor("scx", [128, WP], f32, kind="ExternalInput").ap()
    adjd = nc.dram_tensor("adj", [128, WP], f32, kind="ExternalInput").ap()
    dinid = nc.dram_tensor("dini", [128, WP], f32, kind="ExternalInput").ap()
    lossO = nc.dram_tensor("loss", [BL, 1], f32, kind="ExternalOutput").ap()

    KW = W + 9  # kappa_all width: col w+4 = kappa(w); + batch pad

    with tile.TileContext(nc) as tc, ExitStack() as ctx:
        dpool = ctx.enter_context(tc.tile_pool(name="scr", bufs=1, space="DRAM"))
        scratch = dpool.tile([G, BL, S, CH], bf16)
        cpool = ctx.enter_context(tc.tile_pool(name="const", bufs=1))
        e_all = cpool.tile([128, BL * 128], bf16)
        nc.sync.dma_start(e_all[:], earr[:])
        shE = cpool.tile([128, 128], bf16)
        nc.sync.dma_start(shE[:], shEd[:])
        shK = cpool.tile([128, 128], f32)
        nc.sync.dma_start(shK[:], shKd[:])
        lam1t = cpool.tile([128, WP], f32)
        nc.sync.dma_start(lam1t[:], lam1d[:])
        lam2t = cpool.tile([128, WP], f32)
        nc.sync.dma_start(lam2t[:], lam2d[:])
        scxt = cpool.tile([128, WP], f32)
        nc.sync.dma_start(scxt[:], scxd[:])
        adjt = cpool.tile([128, WP], f32)
        nc.sync.dma_start(adjt[:], adjd[:])
        dinit = cpool.tile([128, WP], f32)
        nc.sync.dma_start(dinit[:], dinid[:])

        # ---------------- Phase 1: gather via PE, store to scratch ---------
        with ExitStack() as p1:
            ypool = p1.enter_context(tc.tile_pool(name="yin", bufs=3))
            psA = p1.enter_context(tc.tile_pool(name="psA", bufs=2, space="PSUM"))
            psB = p1.enter_context(tc.tile_pool(name="psB", bufs=2, space="PSUM"))
            ppool = p1.enter_context(tc.tile_pool(name="pp", bufs=3))
            for b in range(BL):
                yt = ypool.tile([128, T], bf16, tag="y")
                nc.sync.dma_start(yt[:], ytd[b])
                ps0 = psA.tile([128, 512], f32, tag="ps0")
                nc.tensor.matmul(
                    ps0[:], e_all[:, b * 128:(b + 1) * 128], yt[:, 0:512],
                    start=True, stop=True,
                )
                ps1 = psB.tile([128, 512], f32, tag="ps1")
                nc.tensor.matmul(
                    ps1[:], e_all[:, b * 128:(b + 1) * 128], yt[:, 512:1024],
                    start=True, stop=True,
                )
                P = ppool.tile([128, T], bf16, tag="P")
                nc.scalar.copy(P[:, 0:512], ps0[:])
                nc.vector.tensor_scalar(
                    P[:, 512:1024], ps1[:], 1.0, None, Alu.mult,
                )
                for g in range(G):
                    nc.sync.dma_start(
                        scratch[g, b, 0:128, :],
                        P[:, g * CH:(g + 1) * CH],
                    )
                    nc.sync.dma_start(
                        scratch[g, b, 128:129, :],
                        P[0:1, g * CH:(g + 1) * CH],
                    )

        # ---------------- Phase 2: wavefront scan --------------------------
        lpool = ctx.enter_context(tc.tile_pool(name="lp", bufs=1))
        rpool = ctx.enter_context(tc.tile_pool(name="ring", bufs=1))
        kpool = ctx.enter_context(tc.tile_pool(name="kap", bufs=1))
        hpool = ctx.enter_context(tc.tile_pool(name="h", bufs=3))
        xpool = ctx.enter_context(tc.tile_pool(name="x", bufs=3))
        cc1pool = ctx.enter_context(tc.tile_pool(name="cc1", bufs=3))
        cc2pool = ctx.enter_context(tc.tile_pool(name="cc2", bufs=3))
        psP = ctx.enter_context(tc.tile_pool(name="psP", bufs=3, space="PSUM"))
        fpool = ctx.enter_context(tc.tile_pool(name="fin", bufs=1))

        lp = lpool.tile([128, NSLOT * CH], bf16)
        # zero pad slots each group never gets
        for g in range(G):
            lo, hi = SKEW * g, SKEW * g + S
            if lo > 0:
                nc.gpsimd.memset(lp[BL * g:BL * (g + 1), 0:lo * CH], 0.0)
            if hi < NSLOT:
                nc.gpsimd.memset(
                    lp[BL * g:BL * (g + 1), hi * CH:NSLOT * CH], 0.0
                )
        ring = rpool.tile([128, 8 * (CH + 1)], bf16)
        nc.gpsimd.memset(ring[:], 0.0)
        kap = kpool.tile([128, KW], f32)
        nc.gpsimd.memset(kap[:], 0.0)

        # load lp in slot pieces (ascending slots so early waves start sooner)
        for j in range(0, S, 16):
            n = min(16, S - j)
            for g in range(G):
                s0 = SKEW * g + j
                nc.sync.dma_start(
                    lp[BL * g:BL * (g + 1), s0 * CH:(s0 + n) * CH].rearrange(
                        "p (s c) -> p s c", s=n
                    ),
                    scratch[g, :, j:j + n, :],
                )

        ringv = ring[:].rearrange("p (s c) -> p s c", s=8)
        c1_tiles = {}
        c2_tiles = {}

        def emit_batch(w0):
            """Crossing bookkeeping for waves w0..w0+3 (w0 % 4 == 0)."""
            P3 = psP.tile([128, 8], f32, tag="psP")
            if w0 >= KB:
                sl0 = (w0 - SKEW) % 8
                nc.tensor.matmul(
                    P3[:, 0:4], shE[:], ringv[:, sl0:sl0 + 4, CH:CH + 1],
                    start=True, stop=True,
                )
                nc.tensor.matmul(
                    P3[:, 4:8], shK[:], kap[:, w0 - 4:w0],
                    start=True, stop=True,
                )
            else:
                nc.vector.memset(P3[:], 0.0)
            v0 = xpool.tile([128, 4], f32, tag="v0")
            nc.vector.scalar_tensor_tensor(
                v0[:], P3[:, 0:4], 1.0, scxt[:, w0:w0 + 4], Alu.mult, Alu.mult,
            )
            v = xpool.tile([128, 4], f32, tag="v")
            nc.vector.tensor_scalar(
                v[:], v0[:], float(2.0 ** -40), None, Alu.max,
            )
            qb = xpool.tile([128, 4], i32, tag="qb")
            nc.vector.tensor_scalar(
                qb[:], v[:].bitcast(i32), 0x7F800000, None, Alu.bitwise_and,
            )
            vrb = xpool.tile([128, 4], i32, tag="vrb")
            nc.vector.tensor_scalar(
                vrb[:], qb[:], 0x7F800000, None, Alu.bitwise_xor,
            )
            fb = xpool.tile([128, 4], i32, tag="fb")
            nc.vector.tensor_scalar(
                fb[:], qb[:], 23, None, Alu.logical_shift_right,
            )
            u1 = xpool.tile([128, 4], f32, tag="u1")
            nc.vector.scalar_tensor_tensor(
                u1[:], fb[:], 1.0, adjt[:, w0:w0 + 4],
                Alu.mult, Alu.add,
            )
            nc.vector.scalar_tensor_tensor(
                kap[:, w0 + 4:w0 + 8], u1[:], 1.0, P3[:, 4:8],
                Alu.mult, Alu.add,
            )
            i1 = xpool.tile([128, 4], f32, tag="i1")
            nc.vector.scalar_tensor_tensor(
                i1[:], v[:], 1.0, vrb[:].bitcast(f32), Alu.mult, Alu.mult,
            )
            sl = w0 % 8
            nc.vector.scalar_tensor_tensor(
                ringv[:, sl:sl + 4, 0:1], i1[:], 1.0, dinit[:, w0:w0 + 4],
                Alu.mult, Alu.mult,
            )
            # c1 = 2^clamp(kprev1 - knew) * lam1 ; c2 likewise vs kprev2
            d1 = xpool.tile([128, 4], f32, tag="d1")
            nc.vector.scalar_tensor_tensor(
                d1[:], kap[:, w0 + 4:w0 + 8], -1.0, kap[:, w0 + 3:w0 + 7],
                Alu.mult, Alu.add,
            )
            ub1 = xpool.tile([128, 4], f32, tag="ub1")
            nc.vector.tensor_scalar(
                ub1[:], d1[:], 63.0, None, Alu.min,
            )
            nc.vector.tensor_scalar(
                ub1[:], ub1[:], -126.0, FBIAS, Alu.max, Alu.add,
            )
            m1b = xpool.tile([128, 4], i32, tag="m1b")
            nc.vector.tensor_scalar(
                m1b[:], ub1[:].bitcast(i32), 0x1FF, None, Alu.bitwise_and,
            )
            nc.vector.tensor_scalar(
                m1b[:], m1b[:], 23, None, Alu.logical_shift_left,
            )
            c1 = cc1pool.tile([128, 4], f32, tag="c1")
            nc.vector.scalar_tensor_tensor(
                c1[:], m1b[:].bitcast(f32), 1.0, lam1t[:, w0:w0 + 4],
                Alu.mult, Alu.mult,
            )
            d2 = xpool.tile([128, 4], f32, tag="d2")
            nc.vector.scalar_tensor_tensor(
                d2[:], kap[:, w0 + 4:w0 + 8], -1.0, kap[:, w0 + 2:w0 + 6],
                Alu.mult, Alu.add,
            )
            ub2 = xpool.tile([128, 4], f32, tag="ub2")
            nc.vector.tensor_scalar(
                ub2[:], d2[:], 63.0, None, Alu.min,
            )
            nc.vector.tensor_scalar(
                ub2[:], ub2[:], -126.0, FBIAS, Alu.max, Alu.add,
            )
            m2b = xpool.tile([128, 4], i32, tag="m2b")
            nc.vector.tensor_scalar(
                m2b[:], ub2[:].bitcast(i32), 0x1FF, None, Alu.bitwise_and,
            )
            nc.vector.tensor_scalar(
                m2b[:], m2b[:], 23, None, Alu.logical_shift_left,
            )
            c2 = cc2pool.tile([128, 4], f32, tag="c2")
            nc.vector.scalar_tensor_tensor(
                c2[:], m2b[:].bitcast(f32), 1.0, lam2t[:, w0:w0 + 4],
                Alu.mult, Alu.mult,
            )
            c1_tiles[w0] = c1
            c2_tiles[w0] = c2

        emit_batch(0)
        emit_batch(4)
        for w in range(W):
            w0 = (w // KB) * KB
            j = w - w0
            sl = w % 8
            sl1 = (w - 1) % 8
            sl2 = (w - 2) % 8
            base = sl * (CH + 1)
            c1col = c1_tiles[w0][:, j:j + 1]
            c2col = c2_tiles[w0][:, j:j + 1]
            tmp = hpool.tile([128, CH], bf16, tag="tmp")
            nc.gpsimd.tensor_scalar(
                tmp[:], ring[:, sl1 * (CH + 1):sl1 * (CH + 1) + CH], c1col,
                None, Alu.mult,
            )
            h = hpool.tile([128, CH], bf16, tag="h")
            nc.vector.scalar_tensor_tensor(
                h[:], ring[:, sl2 * (CH + 1):sl2 * (CH + 1) + CH], c2col,
                tmp[:], Alu.mult, Alu.add,
            )
            if w < 2:
                initial = float(2.0 ** VG[0][w])
            else:
                initial = ring[:, base:base + 1]
            nc.vector.tensor_tensor_scan(
                ring[:, base + 1:base + CH + 1], h[:],
                lp[:, w * CH:(w + 1) * CH], initial, Alu.add, Alu.mult,
            )
            nw = w + 5
            if nw % KB == 0 and 8 <= nw < W:
                emit_batch(nw)

        # ---------------- Final loss ----------------------------------------
        s127 = (S - 2 + SKEW * 3) % 8          # ring slot of row 127 (wave 151)
        s128 = (S - 1 + SKEW * 3) % 8          # wave 152
        k127 = (S - 2 + SKEW * 3) + 4          # kappa col
        k128 = (S - 1 + SKEW * 3) + 4
        e127 = ring[96:128, s127 * (CH + 1) + CH:s127 * (CH + 1) + CH + 1]
        e128 = ring[96:128, s128 * (CH + 1) + CH:s128 * (CH + 1) + CH + 1]
        ft = fpool.tile([128, 8], f32)
        vdiff = float(VG[3][S - 1] - VG[3][S - 2])
        # d = (k128 - k127) - vdiff ; build 2^d via bit trick
        nc.vector.scalar_tensor_tensor(
            ft[96:128, 0:1], kap[96:128, k127:k127 + 1], -1.0,
            kap[96:128, k128:k128 + 1], Alu.mult, Alu.add,
        )
        nc.vector.tensor_scalar(
            ft[96:128, 1:2], ft[96:128, 0:1], -vdiff + FBIAS, None, Alu.add,
        )
        fti = fpool.tile([128, 8], mybir.dt.int32)
        nc.vector.tensor_scalar(
            fti[96:128, 0:1], ft[96:128, 1:2].bitcast(i32), 0x1FF, None,
            Alu.bitwise_and,
        )
        nc.vector.tensor_scalar(
            fti[96:128, 0:1], fti[96:128, 0:1], 23, None,
            Alu.logical_shift_left,
        )
        nc.vector.scalar_tensor_tensor(
            ft[96:128, 2:3], e128, fti[96:128, 0:1].bitcast(f32), e127,
            Alu.mult, Alu.add,
        )
        nc.scalar.activation(
            ft[96:128, 3:4], ft[96:128, 2:3], mybir.ActivationFunctionType.Ln,
        )
        nc.vector.scalar_tensor_tensor(
            ft[96:128, 4:5], kap[96:128, k127:k127 + 1], LN2,
            ft[96:128, 3:4], Alu.mult, Alu.add,
        )
        ck = -float(VG[3][S - 2] + KTOT) * LN2
        nc.vector.tensor_scalar(
            ft[96:128, 5:6], ft[96:128, 4:5], ck, -1.0, Alu.add, Alu.mult,
        )
        nc.sync.dma_start(lossO[:], ft[96:128, 5:6])

    nc.compile()
    return nc


def _host_prep(y_true, y_pred):
    bf = ml_dtypes.bfloat16
    blank = C - 1
    ext = np.full((B_ALL, S), blank, np.int64)
    ext[:, 1::2] = y_true.astype(np.int64)
    allow = np.zeros((B_ALL, S), np.float64)
    allow[:, 2:] = (
        (ext[:, 2:] != blank) & (ext[:, 2:] != ext[:, :-2])
    ).astype(np.float64)

    ktt = (2.0 ** KT).astype(np.float32)
    # host prescale + transpose + cast (all cores at once)
    ypre = (y_pred + np.float32(EPS)) * ktt[None, :, None]
    ytall = np.ascontiguousarray(ypre.transpose(0, 2, 1)).astype(bf)  # [B,C,T]

    lam1b, lam2base, scx, adj, dini = _build_tables()

    shmat = np.zeros((128, 128), np.float32)
    for i in range(32, 128):
        shmat[i - 32, i] = 1.0

    in_maps = []
    for core in range(8):
        bs = slice(core * BL, (core + 1) * BL)
        exts = ext[bs]
        E = np.zeros((128, BL, 128), np.float32)
        for b in range(BL):
            E[exts[b, :128], b, np.arange(128)] = 1.0
        lam2 = lam2base.copy()
        for p in range(128):
            g = p // BL
            b = p % BL
            for w in range(WP):
                s = w - SKEW * g
                if 2 <= s <= S - 1:
                    lam2[p, w] *= allow[bs][b, s]
        in_maps.append({
            "yt": ytall[bs],
            "earr": E.reshape(128, BL * 128).astype(bf),
            "shE": shmat.astype(bf),
            "shK": shmat,
            "lam1": lam1b,
            "lam2": lam2.astype(np.float32),
            "scx": scx,
            "adj": adj,
            "dini": dini,
        })
    return in_maps


def kernel(y_true, y_pred):
    from concourse.bass_utils import run_bass_kernel_spmd
    y_true = np.asarray(y_true)
    y_pred = np.asarray(y_pred, dtype=np.float32)
    if "nc" not in _cache:
        _cache["nc"] = _build_nc()
    nc = _cache["nc"]
    in_maps = _host_prep(y_true, y_pred)
    res = run_bass_kernel_spmd(nc, in_maps, list(range(8)))
    out = np.concatenate(
        [res.results[i]["loss"].reshape(BL, 1) for i in range(8)], axis=0
    )
    return out.astype(np.float32)


# revision 3
# speedup vs baseline: 2.0520x; 2.0520x over previous
"""CTC loss kernel for Trainium2 (Bass/Tile), 8-core data-parallel.

Two-phase design:
  Phase 1 (gather): per batch row b, one-hot matmul E_b^T @ yT_b on the PE
    gathers extended-label channel probabilities into [state, time] tiles,
    stored to a DRAM scratch in a (group, b, slot, t)-contiguous layout so
    both the store and the reload use large DMA packets. y is pre-scaled on
    host: p'[t] = (y + eps) * 2^kt[t] (global integer drift schedule), bf16.
  Phase 2 (wavefront scan): partitions = 32 b x 4 time-chunks; wave w
    computes row s = w - 8g for chunk g as a hardware scan along time:
    state = (h + state) * p', h = c1*ring[w-1] + c2*ring[w-2].
    Numerical robustness: static per-(chunk,row) scales (VG) folded into
    the mixing tables, plus a per-row dynamic power-of-2 rescale at every
    chunk crossing (integer exponent kappa tracked per partition; extracted
    with f32 bit tricks - no scalar-engine tables). Chunk edges and kappa
    pass between groups via PE shift matmuls, batched 4 waves at a time.
Final: loss = -(ln(e127 + e128*2^d) + kappa127*ln2 - (V127+Ktot)*ln2).
"""
import numpy as np
import ml_dtypes
from contextlib import ExitStack

B_ALL, T, C, L = 256, 1024, 128, 64
S = 2 * L + 1            # 129
BL = B_ALL // 8          # 32 rows per core
G, CH = 4, 256
SKEW = 12
W = S + SKEW * (G - 1)   # 165 waves
NSLOT = W                # lp slots (slot = s + 8g)
KB = 8                   # crossing batch (waves per batch)
WP = ((W + KB - 1) // KB) * KB + KB   # padded table width (160)
EPS = 1e-7
LN2 = float(np.log(2.0))
SLOPE = 1.1
FBIAS = 12582912.0 + 127.0   # 0x4B400000 as f32 is 12582912; +127 exp bias

VG = [[74, 48, 33, 19, 10, 0, -9, -17, -25, -33, -40, -46, -51, -60, -64, -71, -75, -81, -84, -89, -94, -99, -103, -108, -112, -116, -119, -122, -126, -131, -134, -138, -141, -146, -149, -152, -154, -158, -161, -165, -167, -171, -173, -176, -179, -182, -184, -188, -191, -195, -197, -200, -202, -204, -206, -209, -212, -215, -216, -218, -220, -223, -225, -228, -229, -232, -233, -236, -238, -241, -242, -245, -245, -248, -249, -252, -253, -255, -256, -259, -260, -262, -263, -265, -267, -270, -270, -272, -273, -275, -276, -278, -279, -281, -282, -284, -285, -287, -288, -291, -292, -294, -295, -297, -297, -300, -300, -302, -303, -304, -305, -307, -308, -310, -310, -312, -312, -314, -314, -316, -317, -318, -319, -320, -321, -323, -322, -324, -324],
      [160, 117, 96, 81, 68, 52, 41, 27, 18, 6, -3, -12, -19, -28, -34, -43, -49, -58, -64, -71, -78, -86, -91, -97, -103, -110, -116, -122, -127, -135, -139, -145, -149, -156, -160, -165, -170, -175, -179, -184, -188, -193, -197, -201, -205, -210, -214, -219, -223, -227, -230, -234, -237, -240, -245, -250, -253, -257, -260, -265, -268, -271, -275, -278, -282, -285, -288, -293, -296, -299, -302, -305, -308, -312, -314, -318, -321, -324, -326, -330, -332, -336, -339, -341, -345, -347, -350, -354, -357, -360, -362, -366, -368, -371, -373, -377, -379, -382, -385, -388, -390, -393, -395, -398, -400, -402, -405, -408, -410, -412, -415, -418, -419, -421, -424, -427, -429, -431, -433, -437, -438, -440, -442, -444, -446, -448, -450, -452, -454],
      [249, 194, 169, 149, 133, 114, 98, 83, 71, 58, 48, 33, 25, 16, 6, -1, -11, -18, -28, -37, -45, -53, -60, -67, -76, -83, -89, -96, -103, -112, -118, -125, -131, -138, -145, -152, -156, -163, -169, -174, -180, -187, -192, -197, -202, -208, -213, -217, -223, -227, -233, -238, -243, -248, -253, -257, -263, -266, -271, -277, -281, -286, -290, -294, -298, -303, -306, -311, -315, -319, -324, -327, -332, -337, -341, -345, -349, -353, -357, -361, -364, -368, -372, -376, -379, -383, -387, -391, -394, -399, -402, -406, -409, -412, -415, -419, -422, -425, -428, -431, -435, -439, -442, -447, -450, -453, -456, -459, -462, -467, -469, -474, -476, -479, -482, -484, -488, -490, -494, -496, -499, -502, -506, -509, -512, -514, -517, -520, -523],
      [342, 279, 249, 216, 199, 180, 164, 145, 134, 121, 108, 92, 81, 68, 59, 43, 35, 22, 15, 4, -3, -13, -18, -28, -36, -45, -52, -63, -70, -77, -83, -94, -100, -108, -115, -124, -130, -140, -146, -152, -157, -166, -171, -179, -184, -193, -197, -205, -209, -217, -222, -229, -234, -240, -244, -250, -254, -262, -266, -272, -276, -283, -287, -294, -297, -303, -307, -313, -316, -322, -325, -332, -335, -340, -344, -351, -354, -360, -363, -370, -373, -378, -381, -387, -391, -397, -401, -406, -409, -414, -416, -423, -426, -431, -433, -439, -442, -448, -452, -456, -460, -464, -468, -472, -475, -481, -483, -487, -490, -494, -497, -502, -505, -510, -513, -517, -519, -524, -527, -531, -534, -538, -541, -546, -549, -553, -556, -560, -563]]
DG = [[0] * S,
      [-86, -69, -64, -61, -58, -52, -50, -45, -43, -39, -36, -35, -32, -32, -30, -28, -26, -23, -21, -19, -15, -13, -12, -12, -9, -6, -3, 0, 2, 4, 6, 7, 8, 10, 11, 13, 16, 17, 18, 20, 21, 22, 24, 24, 27, 28, 30, 31, 32, 32, 33, 33, 35, 36, 38, 40, 41, 42, 44, 47, 48, 48, 50, 51, 52, 53, 55, 57, 58, 58, 60, 61, 62, 63, 65, 66, 68, 69, 70, 71, 72, 73, 75, 76, 78, 78, 80, 82, 83, 84, 86, 87, 89, 89, 91, 93, 94, 96, 97, 97, 98, 99, 100, 101, 103, 102, 104, 106, 107, 108, 110, 111, 112, 112, 114, 115, 115, 115, 115, 115, 115, 115, 115, 115, 115, 115, 115, 115, 115],
      [-89, -76, -73, -69, -65, -62, -57, -56, -53, -52, -51, -45, -44, -44, -41, -43, -39, -40, -36, -34, -33, -32, -31, -29, -27, -27, -27, -27, -24, -22, -21, -20, -18, -18, -15, -13, -13, -12, -10, -11, -9, -6, -5, -3, -3, -2, -1, -2, 0, 1, 3, 5, 6, 7, 9, 8, 9, 9, 11, 12, 14, 15, 15, 16, 16, 18, 18, 18, 19, 20, 22, 22, 24, 26, 27, 27, 29, 29, 31, 31, 32, 33, 33, 34, 35, 36, 36, 36, 37, 39, 40, 40, 41, 41, 42, 42, 43, 43, 44, 43, 44, 45, 47, 48, 50, 51, 51, 52, 53, 55, 54, 56, 56, 58, 58, 58, 59, 59, 61, 59, 62, 62, 63, 64, 66, 66, 68, 68, 69],
      [-92, -86, -80, -67, -66, -66, -66, -62, -63, -63, -60, -58, -56, -52, -52, -44, -46, -40, -42, -41, -43, -41, -42, -39, -40, -39, -37, -33, -33, -35, -35, -31, -32, -29, -30, -27, -26, -24, -24, -21, -23, -21, -21, -18, -18, -15, -16, -13, -14, -10, -11, -9, -9, -7, -9, -8, -9, -4, -5, -5, -6, -3, -3, 0, -1, 0, 1, 2, 2, 3, 2, 5, 3, 3, 2, 6, 5, 7, 6, 9, 8, 10, 9, 12, 11, 14, 14, 16, 15, 15, 14, 17, 17, 19, 18, 20, 21, 23, 23, 25, 25, 25, 25, 26, 25, 28, 27, 27, 27, 27, 28, 29, 29, 31, 31, 32, 32, 34, 34, 36, 35, 36, 36, 37, 38, 39, 39, 40, 39]]

_KT_CUM = np.round(SLOPE * np.arange(T + 1)).astype(np.int64)
KT = np.diff(_KT_CUM)
KTOT = int(KT.sum())

_cache = {}


def _wave_gs(p, w):
    """(g, s) for partition p at wave w; s may be out of range (garbage)."""
    g = p // BL
    return g, w - SKEW * g


def _build_tables():
    """Static [128, WP] f32 tables for phase 2."""
    lam1 = np.zeros((128, WP), np.float32)
    lam2base = np.zeros((128, WP), np.float64)  # without allow (per-b applied later)
    scx = np.zeros((128, WP), np.float32)       # 2^(dV + D); 0 for g0/garbage
    adj = np.zeros((128, WP), np.float32)
    dini = np.zeros((128, WP), np.float32)      # 2^-D for real g>=1 crossings
    for p in range(128):
        g = p // BL
        for w in range(WP):
            s = w - SKEW * g
            real = 0 <= s <= S - 1
            if real and s >= 1:
                lam1[p, w] = 2.0 ** float(VG[g][s] - VG[g][s - 1])
            if real and s >= 2:
                lam2base[p, w] = 2.0 ** float(VG[g][s] - VG[g][s - 2])
            if real and g >= 1:
                ev = VG[g][s] - VG[g - 1][s] + DG[g][s]
                ev = max(-120, min(120, ev))
                scx[p, w] = 2.0 ** float(ev)
                adj[p, w] = -127.0
                dini[p, w] = 2.0 ** float(-DG[g][s] - 1)
            else:
                # g0 or garbage: v = max(0, 2^-40) -> fb encodes k=-40;
                # adj makes kappa_new = kappa_src (chains through).
                scx[p, w] = 0.0
                adj[p, w] = -87.0
                dini[p, w] = 0.0
    return lam1, lam2base, scx, adj, dini


def _build_nc():
    import concourse.bass as bass
    import concourse.tile as tile
    from concourse import bacc, mybir
    f32 = mybir.dt.float32
    bf16 = mybir.dt.bfloat16
    i32 = mybir.dt.int32
    Alu = mybir.AluOpType

    nc = bacc.Bacc("TRN2", target_bir_lowering=False, debug=False)
    ytd = nc.dram_tensor("yt", [BL, 128, T], bf16, kind="ExternalInput").ap()
    earr = nc.dram_tensor("earr", [128, BL * 128], bf16, kind="ExternalInput").ap()
    shEd = nc.dram_tensor("shE", [128, 128], bf16, kind="ExternalInput").ap()
    shKd = nc.dram_tensor("shK", [128, 128], f32, kind="ExternalInput").ap()
    lam1d = nc.dram_tensor("lam1", [128, WP], f32, kind="ExternalInput").ap()
    lam2d = nc.dram_tensor("lam2", [128, WP], f32, kind="ExternalInput").ap()
    scxd = nc.dram_tensor("scx", [128, WP], f32, kind="ExternalInput").ap()
    adjd = nc.dram_tensor("adj", [128, WP], f32, kind="ExternalInput").ap()
    dinid = nc.dram_tensor("dini", [128, WP], f32, kind="ExternalInput").ap()
    lossO = nc.dram_tensor("loss", [BL, 1], f32, kind="ExternalOutput").ap()

    KW = W + 4 + 2 * KB  # kappa_all width: col w+4 = kappa(w); + batch pad

    with tile.TileContext(nc) as tc, ExitStack() as ctx:
        dpool = ctx.enter_context(tc.tile_pool(name="scr", bufs=1, space="DRAM"))
        scratch = dpool.tile([G, BL, S, CH], bf16)
        cpool = ctx.enter_context(tc.tile_pool(name="const", bufs=1))
        e_all = cpool.tile([128, BL * 128], bf16)
        nc.sync.dma_start(e_all[:], earr[:])
        shE = cpool.tile([128, 128], bf16)
        nc.sync.dma_start(shE[:], shEd[:])
        shK = cpool.tile([128, 128], f32)
        nc.sync.dma_start(shK[:], shKd[:])
        lam1t = cpool.tile([128, WP], f32)
        nc.sync.dma_start(lam1t[:], lam1d[:])
        lam2t = cpool.tile([128, WP], f32)
        nc.sync.dma_start(lam2t[:], lam2d[:])
        scxt = cpool.tile([128, WP], f32)
        nc.sync.dma_start(scxt[:], scxd[:])
        adjt = cpool.tile([128, WP], f32)
        nc.sync.dma_start(adjt[:], adjd[:])
        dinit = cpool.tile([128, WP], f32)
        nc.sync.dma_start(dinit[:], dinid[:])

        # ---------------- Phase 1: gather via PE, store to scratch ---------
        with ExitStack() as p1:
            ypool = p1.enter_context(tc.tile_pool(name="yin", bufs=3))
            psA = p1.enter_context(tc.tile_pool(name="psA", bufs=2, space="PSUM"))
            psB = p1.enter_context(tc.tile_pool(name="psB", bufs=2, space="PSUM"))
            ppool = p1.enter_context(tc.tile_pool(name="pp", bufs=3))
            for b in range(BL):
                yt = ypool.tile([128, T], bf16, tag="y")
                nc.sync.dma_start(yt[:], ytd[b])
                ps0 = psA.tile([128, 512], f32, tag="ps0")
                nc.tensor.matmul(
                    ps0[:], e_all[:, b * 128:(b + 1) * 128], yt[:, 0:512],
                    start=True, stop=True,
                )
                ps1 = psB.tile([128, 512], f32, tag="ps1")
                nc.tensor.matmul(
                    ps1[:], e_all[:, b * 128:(b + 1) * 128], yt[:, 512:1024],
                    start=True, stop=True,
                )
                P = ppool.tile([128, T], bf16, tag="P")
                nc.scalar.copy(P[:, 0:512], ps0[:])
                nc.vector.tensor_scalar(
                    P[:, 512:1024], ps1[:], 1.0, None, Alu.mult,
                )
                for g in range(G):
                    nc.sync.dma_start(
                        scratch[g, b, 0:128, :],
                        P[:, g * CH:(g + 1) * CH],
                    )
                    nc.sync.dma_start(
                        scratch[g, b, 128:129, :],
                        P[0:1, g * CH:(g + 1) * CH],
                    )

        # ---------------- Phase 2: wavefront scan --------------------------
        lpool = ctx.enter_context(tc.tile_pool(name="lp", bufs=1))
        rpool = ctx.enter_context(tc.tile_pool(name="ring", bufs=1))
        kpool = ctx.enter_context(tc.tile_pool(name="kap", bufs=1))
        hpool = ctx.enter_context(tc.tile_pool(name="h", bufs=3))
        xpool = ctx.enter_context(tc.tile_pool(name="x", bufs=3))
        cc1pool = ctx.enter_context(tc.tile_pool(name="cc1", bufs=3))
        cc2pool = ctx.enter_context(tc.tile_pool(name="cc2", bufs=3))
        psP = ctx.enter_context(tc.tile_pool(name="psP", bufs=3, space="PSUM"))
        fpool = ctx.enter_context(tc.tile_pool(name="fin", bufs=1))

        lp = lpool.tile([128, NSLOT * CH], bf16)
        # zero pad slots each group never gets
        for g in range(G):
            lo, hi = SKEW * g, SKEW * g + S
            if lo > 0:
                nc.gpsimd.memset(lp[BL * g:BL * (g + 1), 0:lo * CH], 0.0)
            if hi < NSLOT:
                nc.gpsimd.memset(
                    lp[BL * g:BL * (g + 1), hi * CH:NSLOT * CH], 0.0
                )
        ring = rpool.tile([128, 16 * (CH + 1)], bf16)
        nc.gpsimd.memset(ring[:], 0.0)
        kap = kpool.tile([128, KW], f32)
        nc.gpsimd.memset(kap[:], 0.0)

        # load lp in slot pieces (ascending slots so early waves start sooner)
        for j in range(0, S, 16):
            n = min(16, S - j)
            for g in range(G):
                s0 = SKEW * g + j
                nc.sync.dma_start(
                    lp[BL * g:BL * (g + 1), s0 * CH:(s0 + n) * CH].rearrange(
                        "p (s c) -> p s c", s=n
                    ),
                    scratch[g, :, j:j + n, :],
                )

        ringv = ring[:].rearrange("p (s c) -> p s c", s=16)
        c1_tiles = {}
        c2_tiles = {}

        def emit_batch(w0):
            """Crossing bookkeeping for waves w0..w0+3 (w0 % 4 == 0)."""
            P3 = psP.tile([128, 2 * KB], f32, tag="psP")
            if w0 >= KB:
                sl0 = (w0 - SKEW) % 16
                n1 = min(KB, 16 - sl0)
                nc.tensor.matmul(
                    P3[:, 0:n1], shE[:], ringv[:, sl0:sl0 + n1, CH:CH + 1],
                    start=True, stop=True,
                )
                if n1 < KB:
                    nc.tensor.matmul(
                        P3[:, n1:KB], shE[:],
                        ringv[:, 0:KB - n1, CH:CH + 1],
                        start=True, stop=True,
                    )
                nc.tensor.matmul(
                    P3[:, KB:2 * KB], shK[:],
                    kap[:, w0 + 4 - SKEW:w0 + 4 - SKEW + KB],
                    start=True, stop=True,
                )
            else:
                nc.vector.memset(P3[:], 0.0)
            v0 = xpool.tile([128, KB], f32, tag="v0")
            nc.vector.scalar_tensor_tensor(
                v0[:], P3[:, 0:KB], 1.0, scxt[:, w0:w0 + KB], Alu.mult, Alu.mult,
            )
            v = xpool.tile([128, KB], f32, tag="v")
            nc.vector.tensor_scalar(
                v[:], v0[:], float(2.0 ** -40), None, Alu.max,
            )
            qb = xpool.tile([128, KB], i32, tag="qb")
            nc.vector.tensor_scalar(
                qb[:], v[:].bitcast(i32), 0x7F800000, None, Alu.bitwise_and,
            )
            vrb = xpool.tile([128, KB], i32, tag="vrb")
            nc.vector.tensor_scalar(
                vrb[:], qb[:], 0x7F800000, None, Alu.bitwise_xor,
            )
            fb = xpool.tile([128, KB], i32, tag="fb")
            nc.vector.tensor_scalar(
                fb[:], qb[:], 23, None, Alu.logical_shift_right,
            )
            u1 = xpool.tile([128, KB], f32, tag="u1")
            nc.vector.scalar_tensor_tensor(
                u1[:], fb[:], 1.0, adjt[:, w0:w0 + KB],
                Alu.mult, Alu.add,
            )
            nc.vector.scalar_tensor_tensor(
                kap[:, w0 + 4:w0 + 4 + KB], u1[:], 1.0, P3[:, KB:2 * KB],
                Alu.mult, Alu.add,
            )
            i1 = xpool.tile([128, KB], f32, tag="i1")
            nc.vector.scalar_tensor_tensor(
                i1[:], v[:], 1.0, vrb[:].bitcast(f32), Alu.mult, Alu.mult,
            )
            sl = w0 % 16
            nc.vector.scalar_tensor_tensor(
                ringv[:, sl:sl + KB, 0:1], i1[:], 1.0, dinit[:, w0:w0 + KB],
                Alu.mult, Alu.mult,
            )
            # c1 = 2^clamp(kprev1 - knew) * lam1 ; c2 likewise vs kprev2
            d1 = xpool.tile([128, KB], f32, tag="d1")
            nc.vector.scalar_tensor_tensor(
                d1[:], kap[:, w0 + 4:w0 + 4 + KB], -1.0, kap[:, w0 + 3:w0 + 3 + KB],
                Alu.mult, Alu.add,
            )
            ub1 = xpool.tile([128, KB], f32, tag="ub1")
            nc.vector.tensor_scalar(
                ub1[:], d1[:], 63.0, None, Alu.min,
            )
            nc.vector.tensor_scalar(
                ub1[:], ub1[:], -126.0, FBIAS, Alu.max, Alu.add,
            )
            m1b = xpool.tile([128, KB], i32, tag="m1b")
            nc.vector.tensor_scalar(
                m1b[:], ub1[:].bitcast(i32), 0x1FF, None, Alu.bitwise_and,
            )
            nc.vector.tensor_scalar(
                m1b[:], m1b[:], 23, None, Alu.logical_shift_left,
            )
            c1 = cc1pool.tile([128, KB], f32, tag="c1")
            nc.vector.scalar_tensor_tensor(
                c1[:], m1b[:].bitcast(f32), 1.0, lam1t[:, w0:w0 + KB],
                Alu.mult, Alu.mult,
            )
            d2 = xpool.tile([128, KB], f32, tag="d2")
            nc.vector.scalar_tensor_tensor(
                d2[:], kap[:, w0 + 4:w0 + 4 + KB], -1.0, kap[:, w0 + 2:w0 + 2 + KB],
                Alu.mult, Alu.add,
            )
            ub2 = xpool.tile([128, KB], f32, tag="ub2")
            nc.vector.tensor_scalar(
                ub2[:], d2[:], 63.0, None, Alu.min,
            )
            nc.vector.tensor_scalar(
                ub2[:], ub2[:], -126.0, FBIAS, Alu.max, Alu.add,
            )
            m2b = xpool.tile([128, KB], i32, tag="m2b")
            nc.vector.tensor_scalar(
                m2b[:], ub2[:].bitcast(i32), 0x1FF, None, Alu.bitwise_and,
            )
            nc.vector.tensor_scalar(
                m2b[:], m2b[:], 23, None, Alu.logical_shift_left,
            )
            c2 = cc2pool.tile([128, KB], f32, tag="c2")
            nc.vector.scalar_tensor_tensor(
                c2[:], m2b[:].bitcast(f32), 1.0, lam2t[:, w0:w0 + KB],
                Alu.mult, Alu.mult,
            )
            c1_tiles[w0] = c1
            c2_tiles[w0] = c2

        emit_batch(0)
        for w in range(W):
            w0 = (w // KB) * KB
            j = w - w0
            even = (w % 2 == 0)
            sl = w % 16
            sl1 = (w - 1) % 16
            sl2 = (w - 2) % 16
            base = sl * (CH + 1)
            c1col = c1_tiles[w0][:, j:j + 1]
            c2col = c2_tiles[w0][:, j:j + 1]
            tmp = hpool.tile([128, CH], bf16, tag="tmp")
            nc.scalar.activation(
                tmp[:], ring[:, sl1 * (CH + 1):sl1 * (CH + 1) + CH],
                mybir.ActivationFunctionType.Identity, bias=0.0, scale=c1col,
            )
            if even:
                h = tmp
            else:
                h = hpool.tile([128, CH], bf16, tag="h")
                nc.vector.scalar_tensor_tensor(
                    h[:], ring[:, sl2 * (CH + 1):sl2 * (CH + 1) + CH], c2col,
                    tmp[:], Alu.mult, Alu.add,
                )
            if w < 2:
                initial = float(2.0 ** VG[0][w])
            else:
                initial = ring[:, base:base + 1]
            nc.vector.tensor_tensor_scan(
                ring[:, base + 1:base + CH + 1], h[:],
                lp[:, w * CH:(w + 1) * CH], initial, Alu.add, Alu.mult,
            )
            nw = w + 5
            if nw % KB == 0 and KB <= nw < W:
                emit_batch(nw)

        # ---------------- Final loss ----------------------------------------
        s127 = (S - 2 + SKEW * 3) % 16         # ring slot of row 127
        s128 = (S - 1 + SKEW * 3) % 16
        k127 = (S - 2 + SKEW * 3) + 4          # kappa col
        k128 = (S - 1 + SKEW * 3) + 4
        e127 = ring[96:128, s127 * (CH + 1) + CH:s127 * (CH + 1) + CH + 1]
        e128 = ring[96:128, s128 * (CH + 1) + CH:s128 * (CH + 1) + CH + 1]
        ft = fpool.tile([128, 8], f32)
        vdiff = float(VG[3][S - 1] - VG[3][S - 2])
        # d = (k128 - k127) - vdiff ; build 2^d via bit trick
        nc.vector.scalar_tensor_tensor(
            ft[96:128, 0:1], kap[96:128, k127:k127 + 1], -1.0,
            kap[96:128, k128:k128 + 1], Alu.mult, Alu.add,
        )
        nc.vector.tensor_scalar(
            ft[96:128, 1:2], ft[96:128, 0:1], -vdiff + FBIAS, None, Alu.add,
        )
        fti = fpool.tile([128, 8], mybir.dt.int32)
        nc.vector.tensor_scalar(
            fti[96:128, 0:1], ft[96:128, 1:2].bitcast(i32), 0x1FF, None,
            Alu.bitwise_and,
        )
        nc.vector.tensor_scalar(
            fti[96:128, 0:1], fti[96:128, 0:1], 23, None,
            Alu.logical_shift_left,
        )
        nc.vector.scalar_tensor_tensor(
            ft[96:128, 2:3], e128, fti[96:128, 0:1].bitcast(f32), e127,
            Alu.mult, Alu.add,
        )
        nc.scalar.activation(
            ft[96:128, 3:4], ft[96:128, 2:3], mybir.ActivationFunctionType.Ln,
        )
        nc.vector.scalar_tensor_tensor(
            ft[96:128, 4:5], kap[96:128, k127:k127 + 1], LN2,
            ft[96:128, 3:4], Alu.mult, Alu.add,
        )
        ck = -float(VG[3][S - 2] + KTOT) * LN2
        nc.vector.tensor_scalar(
            ft[96:128, 5:6], ft[96:128, 4:5], ck, -1.0, Alu.add, Alu.mult,
        )
        nc.sync.dma_start(lossO[:], ft[96:128, 5:6])

    nc.compile()
    return nc


def _host_prep(y_true, y_pred):
    bf = ml_dtypes.bfloat16
    blank = C - 1
    ext = np.full((B_ALL, S), blank, np.int64)
    ext[:, 1::2] = y_true.astype(np.int64)
    allow = np.zeros((B_ALL, S), np.float64)
    allow[:, 2:] = (
        (ext[:, 2:] != blank) & (ext[:, 2:] != ext[:, :-2])
    ).astype(np.float64)

    ktt = (2.0 ** KT).astype(np.float32)
    # host prescale + transpose + cast (all cores at once)
    ypre = (y_pred + np.float32(EPS)) * ktt[None, :, None]
    ytall = np.ascontiguousarray(ypre.transpose(0, 2, 1)).astype(bf)  # [B,C,T]

    lam1b, lam2base, scx, adj, dini = _build_tables()

    shmat = np.zeros((128, 128), np.float32)
    for i in range(32, 128):
        shmat[i - 32, i] = 1.0

    in_maps = []
    for core in range(8):
        bs = slice(core * BL, (core + 1) * BL)
        exts = ext[bs]
        E = np.zeros((128, BL, 128), np.float32)
        for b in range(BL):
            E[exts[b, :128], b, np.arange(128)] = 1.0
        lam2 = lam2base.copy()
        for p in range(128):
            g = p // BL
            b = p % BL
            for w in range(WP):
                s = w - SKEW * g
                if 2 <= s <= S - 1:
                    lam2[p, w] *= allow[bs][b, s]
        in_maps.append({
            "yt": ytall[bs],
            "earr": E.reshape(128, BL * 128).astype(bf),
            "shE": shmat.astype(bf),
            "shK": shmat,
            "lam1": lam1b,
            "lam2": lam2.astype(np.float32),
            "scx": scx,
            "adj": adj,
            "dini": dini,
        })
    return in_maps


def kernel(y_true, y_pred):
    from concourse.bass_utils import run_bass_kernel_spmd
    y_true = np.asarray(y_true)
    y_pred = np.asarray(y_pred, dtype=np.float32)
    if "nc" not in _cache:
        _cache["nc"] = _build_nc()
    nc = _cache["nc"]
    in_maps = _host_prep(y_true, y_pred)
    res = run_bass_kernel_spmd(nc, in_maps, list(range(8)))
    out = np.concatenate(
        [res.results[i]["loss"].reshape(BL, 1) for i in range(8)], axis=0
    )
    return out.astype(np.float32)
